# revision 12
# baseline (speedup 1.0000x reference)
"""Channelwise symmetric Hausdorff distance loss on 8 Trainium2 NeuronCores.

Math (per (batch, channel) pair; x, y are [N, D] point sets):
    d2[n, m] = |x_n|^2 + |y_m|^2 - 2 x_n.y_m
    h = max( max_n min_m d(n,m), max_m min_n d(n,m) )
    answer   = mean over the B*C pairs of h.

Sharding: B*C = 24 pairs, 3 per NeuronCore (data parallel), host gathers.

v4 design: the device computes ONLY the -2 x.y products and per-group
row minima; every rank-1 term (|x|^2, |y|^2) is folded in by the host in
float64, so the PE runs nothing but 8 accumulating fp8 DoubleRow matmuls
per n-tile (no fold matmul), ACT does a plain psum->bf16 copy, and the
DVE does one grouped tensor_reduce. y points are pre-sorted by |y|^2 so
the fwd (max_n min_m) direction can use per-group midpoint constants;
measured approximation error is ~1e-4 (tolerance 2e-2). The bwd
direction is computed exactly on host from the full -2xy tiles (bf16,
DMA'd out while the PE computes the next tiles).

Per n-tile: PE 8 matmuls -> ACT copy psum to bf16 cc tile -> DVE
tensor_reduce min over 32-column groups -> cc tile DMA'd to HBM.
"""

import numpy as np

B, C, N, D = 8, 3, 1024, 1024
N_CORES = 8
PAIRS = B * C              # 24
PP = PAIRS // N_CORES      # 3 pairs per core
NT = N // 128              # 8 n-tiles (output partition dim)
MBS = 512                  # m block size (one PSUM bank of fp32)
MB = N // MBS              # 2 m-blocks
KT = D // 128              # 8 k-tiles (contraction)
G = 32                     # y2-sorted groups for the fwd reduce
GS = N // G                # group size

_NC_CACHE = None


def _legalize_sync(nc):
    """This toolchain's walrus accepts at most ONE sync-wait per instruction;
    Tile emits several (e.g. the tail drain waits on every engine/DMA sem).
    Hoist all but the last wait of each instruction into standalone
    InstEventSemaphore instructions on the same engine, inserted just before
    it — semantically identical (the engine blocks on each in turn)."""
    import concourse.mybir as mybir

    n_split = 0
    for fn in nc.m.functions:
        for bb in fn.blocks:
            new_il = []
            for ins in bb.instructions:
                si = ins.sync_info
                if si is not None and si.on_wait and len(si.on_wait) > 1:
                    waits = list(si.on_wait)
                    for k, w in enumerate(waits[:-1]):
                        ev = mybir.InstEventSemaphore(
                            name=f"{ins.name}-evw{k}",
                            engine=ins.engine,
                            ins=[],
                            outs=[],
                            sync_info=mybir.SyncInfo(on_wait=[w], on_update=[]),
                        )
                        new_il.append(ev)
                        n_split += 1
                    si.on_wait = [waits[-1]]
                new_il.append(ins)
            bb.instructions[:] = new_il
    return n_split


def _build_nc():
    import concourse.bass as bass
    import concourse.mybir as mybir
    import concourse.tile as tile

    bf16 = mybir.dt.bfloat16
    f32 = mybir.dt.float32
    f8 = mybir.dt.float8e4
    op_min = mybir.AluOpType.min

    nc = bass.Bass("TRN2", target_bir_lowering=True, debug=False)
    # partition-major host layouts: xt[j, p, k, n] = -2 x[j, n, 128k+p]
    xt_d = nc.dram_tensor("xt", [PP, 128, KT, N], f8, kind="ExternalInput").ap()
    yt_d = nc.dram_tensor("yt", [PP, 128, KT, N], f8, kind="ExternalInput").ap()
    x2c_d = nc.dram_tensor("x2c", [128, PP * NT], f32, kind="ExternalInput").ap()
    row_d = nc.dram_tensor("rowout", [PP, 128, NT * G], f32, kind="ExternalOutput").ap()
    col_d = nc.dram_tensor("colout", [PP, 128, MBS], bf16, kind="ExternalOutput").ap()
    cc_d = nc.dram_tensor("ccout", [PP, NT, 128, MBS], bf16, kind="ExternalOutput").ap()

    with tile.TileContext(nc) as tc:
        with (
            tc.tile_pool(name="xy", bufs=2) as xy_pool,
            tc.tile_pool(name="small", bufs=2) as small_pool,
            tc.tile_pool(name="cc", bufs=8) as cc_pool,
            tc.tile_pool(name="ps", bufs=4, space="PSUM") as ps_pool,
        ):
            x2c_sb = small_pool.tile([128, PP * NT], f32, tag="x2c")
            nc.scalar.dma_start(out=x2c_sb, in_=x2c_d)

            # PE warmup: dataless matmuls bridge the ~5us first-chunk DMA
            # wait so the HAM clock-gate is already released (and the PE
            # never idles >3.4us) when the real stream begins.
            wu = small_pool.tile([128, 2, MBS], f8, tag="wu")
            nc.vector.memset(wu, 1.0)
            wu_ps = ps_pool.tile([128, MB, MBS], f32, tag="ps")
            for i in range(13):
                nc.tensor.matmul(
                    wu_ps[:, 0, :],
                    wu[:, :, 0:128],
                    wu,
                    start=(i == 0),
                    stop=(i == 12),
                    perf_mode=mybir.MatmulPerfMode.DoubleRow,
                )

            # One tile + one DMA per (tensor, k-chunk-pair): chunk-level
            # matmul deps AND queue-level transfer parallelism. xt issues on
            # Sync, yt on GpSimd.
            xt_c, yt_c = [], []
            for j in range(PP):
                xc, yc = [], []
                for k2 in range(KT // 2):
                    xk = xy_pool.tile([128, 2, N], f8, tag=f"xt{k2}")
                    yk = xy_pool.tile([128, 2, N], f8, tag=f"yt{k2}")
                    nc.sync.dma_start(out=xk, in_=xt_d[j, :, 2 * k2 : 2 * k2 + 2, :])
                    nc.gpsimd.dma_start(out=yk, in_=yt_d[j, :, 2 * k2 : 2 * k2 + 2, :])
                    xc.append(xk)
                    yc.append(yk)
                xt_c.append(xc)
                yt_c.append(yc)

            for j in range(PP):
                rowaccs = small_pool.tile([128, NT * G], f32, tag="rowaccs")
                colacc = small_pool.tile([128, MBS], bf16, tag="colacc")
                for nt in range(NT):
                    nsl = slice(nt * 128, (nt + 1) * 128)
                    ps = ps_pool.tile([128, MB, MBS], f32, tag="ps")
                    for ki in range(KT // 2):
                        xsl = xt_c[j][ki][:, :, nsl]
                        yt_k = yt_c[j][ki]
                        for mb in range(MB):
                            nc.tensor.matmul(
                                ps[:, mb, :],
                                xsl,
                                yt_k[:, :, mb * MBS : (mb + 1) * MBS],
                                start=(ki == 0),
                                stop=(ki == KT // 2 - 1),
                                perf_mode=mybir.MatmulPerfMode.DoubleRow,
                            )
                    # ACT evacuates psum + x2c[n] (-> x2 - 1024 - 2xy) bf16
                    cc = cc_pool.tile([128, N], bf16, tag="cc")
                    nc.scalar.activation(
                        cc.rearrange("p (a m) -> p a m", a=MB),
                        ps,
                        mybir.ActivationFunctionType.Identity,
                        bias=x2c_sb[:, j * NT + nt : j * NT + nt + 1],
                        scale=1.0,
                    )
                    # fwd: per-group min over the y2-sorted columns
                    nc.vector.tensor_reduce(
                        out=rowaccs[:, nt * G : (nt + 1) * G],
                        in_=cc.rearrange("p (g m) -> p g m", g=G),
                        axis=mybir.AxisListType.X,
                        op=op_min,
                    )
                    # bwd m<512: on-chip min across n-tiles (DVE has slack)
                    if nt == 0:
                        nc.vector.tensor_copy(colacc, cc[:, 0:MBS])
                    else:
                        nc.vector.tensor_tensor(
                            out=colacc, in0=cc[:, 0:MBS], in1=colacc, op=op_min
                        )
                    # bwd m>=512: tile to HBM, host finishes; alternate the
                    # issue queue-engine so neither queue serializes.
                    eng = nc.sync if nt % 2 == 0 else nc.gpsimd
                    eng.dma_start(out=cc_d[j, nt], in_=cc[:, MBS:N])
                nc.scalar.dma_start(out=col_d[j], in_=colacc)
                nc.scalar.dma_start(out=row_d[j], in_=rowaccs)
                nc.scalar.dma_start(out=row_d[j], in_=rowaccs)
    _legalize_sync(nc)
    return nc


def _prep_inputs(x, y):
    import ml_dtypes

    f8np = np.dtype(ml_dtypes.float8_e4m3)
    x32 = np.ascontiguousarray(x, dtype=np.float32).reshape(PAIRS, N, D)
    y32 = np.ascontiguousarray(y, dtype=np.float32).reshape(PAIRS, N, D)

    x2 = np.square(x32.astype(np.float64)).sum(-1)  # [PAIRS, N]
    y2 = np.square(y32.astype(np.float64)).sum(-1)

    # Sort each pair's y points by |y|^2 so the fwd reduce can use
    # per-group midpoint constants with small within-group span.
    orders = np.argsort(y2, axis=1)
    y2s = np.take_along_axis(y2, orders, axis=1)  # [PAIRS, N] sorted

    # xt[q, p, k, n] = -2 x[q, n, 128k+p]; yt[q, p, k, m] = ys[q, m, 128k+p]
    xt8 = np.empty((PAIRS, 128, KT, N), f8np)
    yt8 = np.empty((PAIRS, 128, KT, N), f8np)
    for q in range(PAIRS):
        xt8[q] = (
            (x32[q].T * np.float32(-2.0)).reshape(KT, 128, N).transpose(1, 0, 2)
        ).astype(f8np)
        ys = y32[q][orders[q]]
        yt8[q] = (ys.T.reshape(KT, 128, N).transpose(1, 0, 2)).astype(f8np)

    y2grp = y2s.reshape(PAIRS, G, GS)
    cg = (y2grp.min(-1) + y2grp.max(-1)) / 2.0  # [PAIRS, G] midpoints

    # x2c[core][p, j*NT + t] = x2[q0+j, t*128 + p] - 1024  (fp32)
    x2c_pairs = (
        (x2 - 1024.0).reshape(PAIRS, NT, 128).transpose(0, 2, 1).astype(np.float32)
    )
    x2c = np.ascontiguousarray(
        x2c_pairs.reshape(N_CORES, PP, 128, NT).transpose(0, 2, 1, 3).reshape(
            N_CORES, 128, PP * NT
        )
    )
    return xt8, yt8, x2, y2s, cg, x2c


def _run(x, y, trace=False):
    global _NC_CACHE
    from concourse.bass_utils import run_bass_kernel_spmd

    xt8, yt8, x2, y2s, cg, x2c = _prep_inputs(x, y)

    if _NC_CACHE is None:
        _NC_CACHE = _build_nc()
    nc = _NC_CACHE

    in_maps = []
    for i in range(N_CORES):
        q0 = i * PP
        in_maps.append(
            {"xt": xt8[q0 : q0 + PP], "yt": yt8[q0 : q0 + PP], "x2c": x2c[i]}
        )

    res = run_bass_kernel_spmd(nc, in_maps, core_ids=list(range(N_CORES)), trace=trace)

    h2 = np.empty(PAIRS, np.float64)
    for i in range(N_CORES):
        r = res.results[i]
        for j in range(PP):
            q = i * PP + j
            # rowaccs[p, t*G+g] = min_{m in grp g}(x2 - 1024 - 2xy)
            rg = r["rowout"][j].astype(np.float64).reshape(128, NT, G)
            rn = (rg + cg[q][None, None, :]).min(-1)  # [128, NT] ~ min_m(d2)-1024
            fwd2 = rn.max() + 1024.0
            # bwd: m<512 from on-chip colacc, m>=512 from raw cc tiles
            colacc = r["colout"][j].astype(np.float64)  # [128, 512]
            lo = colacc.min(0)  # [512] = min_n(x2 - 1024 - 2xy)
            cc = r["ccout"][j].astype(np.float32).reshape(N, MBS)  # [n, m-512]
            hi = cc.min(0).astype(np.float64)
            colmin = np.concatenate([lo, hi])
            bwd2 = (colmin + y2s[q]).max() + 1024.0
            h2[q] = max(fwd2, bwd2, 0.0)

    ans = np.sqrt(h2).mean()
    return np.array(ans, dtype=np.float32), res


def kernel(input, target):
    out, _ = _run(np.asarray(input), np.asarray(target), trace=False)
    return out


# revision 13
# speedup vs baseline: 1.0992x; 1.0992x over previous
"""Channelwise symmetric Hausdorff distance loss on 8 Trainium2 NeuronCores.

Math (per (batch, channel) pair; x, y are [N, D] point sets):
    d2[n, m] = |x_n|^2 + |y_m|^2 - 2 x_n.y_m
    h = max( max_n min_m d(n,m), max_m min_n d(n,m) )
    answer   = mean over the B*C pairs of h.

Sharding: B*C = 24 pairs, 3 per NeuronCore (data parallel), host gathers.

v4 design: the device computes ONLY the -2 x.y products and per-group
row minima; every rank-1 term (|x|^2, |y|^2) is folded in by the host in
float64, so the PE runs nothing but 8 accumulating fp8 DoubleRow matmuls
per n-tile (no fold matmul), ACT does a plain psum->bf16 copy, and the
DVE does one grouped tensor_reduce. y points are pre-sorted by |y|^2 so
the fwd (max_n min_m) direction can use per-group midpoint constants;
measured approximation error is ~1e-4 (tolerance 2e-2). The bwd
direction is computed exactly on host from the full -2xy tiles (bf16,
DMA'd out while the PE computes the next tiles).

Per n-tile: PE 8 matmuls -> ACT copy psum to bf16 cc tile -> DVE
tensor_reduce min over 32-column groups -> cc tile DMA'd to HBM.
"""

import numpy as np

B, C, N, D = 8, 3, 1024, 1024
N_CORES = 8
PAIRS = B * C              # 24
PP = PAIRS // N_CORES      # 3 pairs per core
NT = N // 128              # 8 n-tiles (output partition dim)
MBS = 512                  # m block size (one PSUM bank of fp32)
MB = N // MBS              # 2 m-blocks
KT = D // 128              # 8 k-tiles (contraction)
G = 32                     # y2-sorted groups for the fwd reduce
GS = N // G                # group size

_NC_CACHE = None


def _legalize_sync(nc):
    """This toolchain's walrus accepts at most ONE sync-wait per instruction;
    Tile emits several (e.g. the tail drain waits on every engine/DMA sem).
    Hoist all but the last wait of each instruction into standalone
    InstEventSemaphore instructions on the same engine, inserted just before
    it — semantically identical (the engine blocks on each in turn)."""
    import concourse.mybir as mybir

    n_split = 0
    for fn in nc.m.functions:
        for bb in fn.blocks:
            new_il = []
            for ins in bb.instructions:
                si = ins.sync_info
                if si is not None and si.on_wait and len(si.on_wait) > 1:
                    waits = list(si.on_wait)
                    for k, w in enumerate(waits[:-1]):
                        ev = mybir.InstEventSemaphore(
                            name=f"{ins.name}-evw{k}",
                            engine=ins.engine,
                            ins=[],
                            outs=[],
                            sync_info=mybir.SyncInfo(on_wait=[w], on_update=[]),
                        )
                        new_il.append(ev)
                        n_split += 1
                    si.on_wait = [waits[-1]]
                new_il.append(ins)
            bb.instructions[:] = new_il
    return n_split


def _build_nc():
    import concourse.bass as bass
    import concourse.mybir as mybir
    import concourse.tile as tile

    bf16 = mybir.dt.bfloat16
    f32 = mybir.dt.float32
    f8 = mybir.dt.float8e4
    op_min = mybir.AluOpType.min

    nc = bass.Bass("TRN2", target_bir_lowering=True, debug=False)
    # partition-major host layouts: xt[j, p, k, n] = -2 x[j, n, 128k+p]
    xt_d = nc.dram_tensor("xt", [PP, 128, KT, N], f8, kind="ExternalInput").ap()
    yt_d = nc.dram_tensor("yt", [PP, 128, KT, N], f8, kind="ExternalInput").ap()
    x2c_d = nc.dram_tensor("x2c", [128, PP * NT], f32, kind="ExternalInput").ap()
    row_d = nc.dram_tensor("rowout", [PP, 128, NT * G], f32, kind="ExternalOutput").ap()
    col_d = nc.dram_tensor("colout", [PP, 128, MBS], bf16, kind="ExternalOutput").ap()
    cc_d = nc.dram_tensor("ccout", [PP, NT, 128, MBS], bf16, kind="ExternalOutput").ap()

    with tile.TileContext(nc) as tc:
        with (
            tc.tile_pool(name="xy", bufs=2) as xy_pool,
            tc.tile_pool(name="small", bufs=2) as small_pool,
            tc.tile_pool(name="cc", bufs=8) as cc_pool,
            tc.tile_pool(name="ps", bufs=4, space="PSUM") as ps_pool,
        ):
            x2c_sb = small_pool.tile([128, PP * NT], f32, tag="x2c")
            nc.scalar.dma_start(out=x2c_sb, in_=x2c_d)

            # One tile + one DMA per (tensor, k-chunk-pair): chunk-level
            # matmul deps AND queue-level transfer parallelism. xt issues on
            # Sync, yt on GpSimd.
            xt_c, yt_c = [], []
            for j in range(PP):
                xc, yc = [], []
                for k2 in range(KT // 2):
                    xk = xy_pool.tile([128, 2, N], f8, tag=f"xt{k2}")
                    yk = xy_pool.tile([128, 2, N], f8, tag=f"yt{k2}")
                    nc.sync.dma_start(out=xk, in_=xt_d[j, :, 2 * k2 : 2 * k2 + 2, :])
                    nc.gpsimd.dma_start(out=yk, in_=yt_d[j, :, 2 * k2 : 2 * k2 + 2, :])
                    xc.append(xk)
                    yc.append(yk)
                xt_c.append(xc)
                yt_c.append(yc)

            for j in range(PP):
                rowaccs = small_pool.tile([128, NT * G], f32, tag="rowaccs")
                colacc = small_pool.tile([128, MBS], bf16, tag="colacc")
                for nt in range(NT):
                    nsl = slice(nt * 128, (nt + 1) * 128)
                    ps = ps_pool.tile([128, MB, MBS], f32, tag="ps")
                    for ki in range(KT // 2):
                        xsl = xt_c[j][ki][:, :, nsl]
                        yt_k = yt_c[j][ki]
                        for mb in range(MB):
                            nc.tensor.matmul(
                                ps[:, mb, :],
                                xsl,
                                yt_k[:, :, mb * MBS : (mb + 1) * MBS],
                                start=(ki == 0),
                                stop=(ki == KT // 2 - 1),
                                perf_mode=mybir.MatmulPerfMode.DoubleRow,
                            )
                    # ACT evacuates psum + x2c[n] (-> x2 - 1024 - 2xy) bf16
                    cc = cc_pool.tile([128, N], bf16, tag="cc")
                    nc.scalar.activation(
                        cc.rearrange("p (a m) -> p a m", a=MB),
                        ps,
                        mybir.ActivationFunctionType.Identity,
                        bias=x2c_sb[:, j * NT + nt : j * NT + nt + 1],
                        scale=1.0,
                    )
                    # fwd: per-group min over the y2-sorted columns
                    nc.vector.tensor_reduce(
                        out=rowaccs[:, nt * G : (nt + 1) * G],
                        in_=cc.rearrange("p (g m) -> p g m", g=G),
                        axis=mybir.AxisListType.X,
                        op=op_min,
                    )
                    # bwd m<512: on-chip min across n-tiles (DVE has slack)
                    if nt == 0:
                        nc.vector.tensor_copy(colacc, cc[:, 0:MBS])
                    else:
                        nc.vector.tensor_tensor(
                            out=colacc, in0=cc[:, 0:MBS], in1=colacc, op=op_min
                        )
                    # bwd m>=512: tile to HBM, host finishes; alternate the
                    # issue queue-engine so neither queue serializes.
                    eng = nc.sync if nt % 2 == 0 else nc.gpsimd
                    eng.dma_start(out=cc_d[j, nt], in_=cc[:, MBS:N])
                nc.scalar.dma_start(out=col_d[j], in_=colacc)
                nc.scalar.dma_start(out=row_d[j], in_=rowaccs)
                nc.scalar.dma_start(out=row_d[j], in_=rowaccs)
    _legalize_sync(nc)
    return nc


def _prep_inputs(x, y):
    import ml_dtypes

    f8np = np.dtype(ml_dtypes.float8_e4m3)
    x32 = np.ascontiguousarray(x, dtype=np.float32).reshape(PAIRS, N, D)
    y32 = np.ascontiguousarray(y, dtype=np.float32).reshape(PAIRS, N, D)

    x2 = np.square(x32.astype(np.float64)).sum(-1)  # [PAIRS, N]
    y2 = np.square(y32.astype(np.float64)).sum(-1)

    # Sort each pair's y points by |y|^2 so the fwd reduce can use
    # per-group midpoint constants with small within-group span.
    orders = np.argsort(y2, axis=1)
    y2s = np.take_along_axis(y2, orders, axis=1)  # [PAIRS, N] sorted

    # xt[q, p, k, n] = -2 x[q, n, 128k+p]; yt[q, p, k, m] = ys[q, m, 128k+p]
    xt8 = np.empty((PAIRS, 128, KT, N), f8np)
    yt8 = np.empty((PAIRS, 128, KT, N), f8np)
    for q in range(PAIRS):
        xt8[q] = (
            (x32[q].T * np.float32(-2.0)).reshape(KT, 128, N).transpose(1, 0, 2)
        ).astype(f8np)
        ys = y32[q][orders[q]]
        yt8[q] = (ys.T.reshape(KT, 128, N).transpose(1, 0, 2)).astype(f8np)

    y2grp = y2s.reshape(PAIRS, G, GS)
    cg = (y2grp.min(-1) + y2grp.max(-1)) / 2.0  # [PAIRS, G] midpoints

    # x2c[core][p, j*NT + t] = x2[q0+j, t*128 + p] - 1024  (fp32)
    x2c_pairs = (
        (x2 - 1024.0).reshape(PAIRS, NT, 128).transpose(0, 2, 1).astype(np.float32)
    )
    x2c = np.ascontiguousarray(
        x2c_pairs.reshape(N_CORES, PP, 128, NT).transpose(0, 2, 1, 3).reshape(
            N_CORES, 128, PP * NT
        )
    )
    return xt8, yt8, x2, y2s, cg, x2c


def _run(x, y, trace=False):
    global _NC_CACHE
    from concourse.bass_utils import run_bass_kernel_spmd

    xt8, yt8, x2, y2s, cg, x2c = _prep_inputs(x, y)

    if _NC_CACHE is None:
        _NC_CACHE = _build_nc()
    nc = _NC_CACHE

    in_maps = []
    for i in range(N_CORES):
        q0 = i * PP
        in_maps.append(
            {"xt": xt8[q0 : q0 + PP], "yt": yt8[q0 : q0 + PP], "x2c": x2c[i]}
        )

    res = run_bass_kernel_spmd(nc, in_maps, core_ids=list(range(N_CORES)), trace=trace)

    h2 = np.empty(PAIRS, np.float64)
    for i in range(N_CORES):
        r = res.results[i]
        for j in range(PP):
            q = i * PP + j
            # rowaccs[p, t*G+g] = min_{m in grp g}(x2 - 1024 - 2xy)
            rg = r["rowout"][j].astype(np.float64).reshape(128, NT, G)
            rn = (rg + cg[q][None, None, :]).min(-1)  # [128, NT] ~ min_m(d2)-1024
            fwd2 = rn.max() + 1024.0
            # bwd: m<512 from on-chip colacc, m>=512 from raw cc tiles
            colacc = r["colout"][j].astype(np.float64)  # [128, 512]
            lo = colacc.min(0)  # [512] = min_n(x2 - 1024 - 2xy)
            cc = r["ccout"][j].astype(np.float32).reshape(N, MBS)  # [n, m-512]
            hi = cc.min(0).astype(np.float64)
            colmin = np.concatenate([lo, hi])
            bwd2 = (colmin + y2s[q]).max() + 1024.0
            h2[q] = max(fwd2, bwd2, 0.0)

    ans = np.sqrt(h2).mean()
    return np.array(ans, dtype=np.float32), res


def kernel(input, target):
    out, _ = _run(np.asarray(input), np.asarray(target), trace=False)
    return out


# revision 14
# speedup vs baseline: 1.1673x; 1.0619x over previous
"""Channelwise symmetric Hausdorff distance loss on 8 Trainium2 NeuronCores.

Math (per (batch, channel) pair; x, y are [N, D] point sets):
    d2[n, m] = |x_n|^2 + |y_m|^2 - 2 x_n.y_m
    h = max( max_n min_m d(n,m), max_m min_n d(n,m) )
    answer   = mean over the B*C pairs of h.

Sharding: B*C = 24 pairs, 3 per NeuronCore (data parallel), host gathers.

v4 design: the device computes ONLY the -2 x.y products and per-group
row minima; every rank-1 term (|x|^2, |y|^2) is folded in by the host in
float64, so the PE runs nothing but 8 accumulating fp8 DoubleRow matmuls
per n-tile (no fold matmul), ACT does a plain psum->bf16 copy, and the
DVE does one grouped tensor_reduce. y points are pre-sorted by |y|^2 so
the fwd (max_n min_m) direction can use per-group midpoint constants;
measured approximation error is ~1e-4 (tolerance 2e-2). The bwd
direction is computed exactly on host from the full -2xy tiles (bf16,
DMA'd out while the PE computes the next tiles).

Per n-tile: PE 8 matmuls -> ACT copy psum to bf16 cc tile -> DVE
tensor_reduce min over 32-column groups -> cc tile DMA'd to HBM.
"""

import numpy as np

B, C, N, D = 8, 3, 1024, 1024
N_CORES = 8
PAIRS = B * C              # 24
PP = PAIRS // N_CORES      # 3 pairs per core
NT = N // 128              # 8 n-tiles (output partition dim)
MBS = 512                  # m block size (one PSUM bank of fp32)
MB = N // MBS              # 2 m-blocks
KT = D // 128              # 8 k-tiles (contraction)
G = 32                     # y2-sorted groups for the fwd reduce
GS = N // G                # group size

_NC_CACHE = None


def _legalize_sync(nc):
    """This toolchain's walrus accepts at most ONE sync-wait per instruction;
    Tile emits several (e.g. the tail drain waits on every engine/DMA sem).
    Hoist all but the last wait of each instruction into standalone
    InstEventSemaphore instructions on the same engine, inserted just before
    it — semantically identical (the engine blocks on each in turn)."""
    import concourse.mybir as mybir

    n_split = 0
    for fn in nc.m.functions:
        for bb in fn.blocks:
            new_il = []
            for ins in bb.instructions:
                si = ins.sync_info
                if si is not None and si.on_wait and len(si.on_wait) > 1:
                    waits = list(si.on_wait)
                    for k, w in enumerate(waits[:-1]):
                        ev = mybir.InstEventSemaphore(
                            name=f"{ins.name}-evw{k}",
                            engine=ins.engine,
                            ins=[],
                            outs=[],
                            sync_info=mybir.SyncInfo(on_wait=[w], on_update=[]),
                        )
                        new_il.append(ev)
                        n_split += 1
                    si.on_wait = [waits[-1]]
                new_il.append(ins)
            bb.instructions[:] = new_il
    return n_split


def _build_nc():
    import concourse.bass as bass
    import concourse.mybir as mybir
    import concourse.tile as tile

    bf16 = mybir.dt.bfloat16
    f32 = mybir.dt.float32
    f8 = mybir.dt.float8e4
    op_min = mybir.AluOpType.min

    nc = bass.Bass("TRN2", target_bir_lowering=True, debug=False)
    # partition-major host layouts: xt[j, p, k, n] = -2 x[j, n, 128k+p]
    xt_d = nc.dram_tensor("xt", [PP, 128, KT, N], f8, kind="ExternalInput").ap()
    yt_d = nc.dram_tensor("yt", [PP, 128, KT, N], f8, kind="ExternalInput").ap()
    x2c_d = nc.dram_tensor("x2c", [128, PP * NT], f32, kind="ExternalInput").ap()
    row_d = nc.dram_tensor("rowout", [PP, 128, NT * G], f32, kind="ExternalOutput").ap()
    col_d = nc.dram_tensor("colout", [PP, 128, MBS], bf16, kind="ExternalOutput").ap()
    cc_d = nc.dram_tensor("ccout", [PP, NT, 128, MBS], bf16, kind="ExternalOutput").ap()

    with tile.TileContext(nc) as tc:
        with (
            tc.tile_pool(name="xy", bufs=2) as xy_pool,
            tc.tile_pool(name="small", bufs=2) as small_pool,
            tc.tile_pool(name="cc", bufs=8) as cc_pool,
            tc.tile_pool(name="ps", bufs=4, space="PSUM") as ps_pool,
        ):
            x2c_sb = small_pool.tile([128, PP * NT], f32, tag="x2c")
            nc.scalar.dma_start(out=x2c_sb, in_=x2c_d)

            # One tile + one DMA per (tensor, k-chunk-pair): chunk-level
            # matmul deps AND queue-level transfer parallelism. xt issues on
            # Sync, yt on GpSimd.
            xt_c, yt_c = [], []
            for j in range(PP):
                xc, yc = [], []
                for k2 in range(KT // 2):
                    xk = xy_pool.tile([128, 2, N], f8, tag=f"xt{k2}")
                    yk = xy_pool.tile([128, 2, N], f8, tag=f"yt{k2}")
                    nc.sync.dma_start(out=xk, in_=xt_d[j, :, 2 * k2 : 2 * k2 + 2, :])
                    nc.gpsimd.dma_start(out=yk, in_=yt_d[j, :, 2 * k2 : 2 * k2 + 2, :])
                    xc.append(xk)
                    yc.append(yk)
                xt_c.append(xc)
                yt_c.append(yc)

            for j in range(PP):
                rowaccs = small_pool.tile([128, NT * G], f32, tag="rowaccs")
                colacc = small_pool.tile([128, MBS], bf16, tag="colacc")
                for nt in range(NT):
                    nsl = slice(nt * 128, (nt + 1) * 128)
                    ps = ps_pool.tile([128, MB, MBS], f32, tag="ps")
                    for ki in range(KT // 2):
                        xsl = xt_c[j][ki][:, :, nsl]
                        yt_k = yt_c[j][ki]
                        for mb in range(MB):
                            nc.tensor.matmul(
                                ps[:, mb, :],
                                xsl,
                                yt_k[:, :, mb * MBS : (mb + 1) * MBS],
                                start=(ki == 0),
                                stop=(ki == KT // 2 - 1),
                                perf_mode=mybir.MatmulPerfMode.DoubleRow,
                            )
                    # ACT evacuates psum + x2c[n] (-> x2 - 1024 - 2xy) bf16
                    cc = cc_pool.tile([128, N], bf16, tag="cc")
                    nc.scalar.activation(
                        cc.rearrange("p (a m) -> p a m", a=MB),
                        ps,
                        mybir.ActivationFunctionType.Identity,
                        bias=x2c_sb[:, j * NT + nt : j * NT + nt + 1],
                        scale=1.0,
                    )
                    # fwd: per-group min over the y2-sorted columns
                    nc.vector.tensor_reduce(
                        out=rowaccs[:, nt * G : (nt + 1) * G],
                        in_=cc.rearrange("p (g m) -> p g m", g=G),
                        axis=mybir.AxisListType.X,
                        op=op_min,
                    )
                    # bwd m<512: on-chip min across n-tiles (DVE has slack)
                    if nt == 0:
                        nc.vector.tensor_copy(colacc, cc[:, 0:MBS])
                    else:
                        nc.vector.tensor_tensor(
                            out=colacc, in0=cc[:, 0:MBS], in1=colacc, op=op_min
                        )
                    # bwd m>=512: tile to HBM, host finishes; alternate the
                    # issue queue-engine so neither queue serializes.
                    eng = nc.sync if nt % 2 == 0 else nc.gpsimd
                    eng.dma_start(out=cc_d[j, nt], in_=cc[:, MBS:N])
                nc.scalar.dma_start(out=col_d[j], in_=colacc)
                nc.scalar.dma_start(out=row_d[j], in_=rowaccs)
    _legalize_sync(nc)
    return nc


def _prep_inputs(x, y):
    import ml_dtypes

    f8np = np.dtype(ml_dtypes.float8_e4m3)
    x32 = np.ascontiguousarray(x, dtype=np.float32).reshape(PAIRS, N, D)
    y32 = np.ascontiguousarray(y, dtype=np.float32).reshape(PAIRS, N, D)

    x2 = np.square(x32.astype(np.float64)).sum(-1)  # [PAIRS, N]
    y2 = np.square(y32.astype(np.float64)).sum(-1)

    # Sort each pair's y points by |y|^2 so the fwd reduce can use
    # per-group midpoint constants with small within-group span.
    orders = np.argsort(y2, axis=1)
    y2s = np.take_along_axis(y2, orders, axis=1)  # [PAIRS, N] sorted

    # xt[q, p, k, n] = -2 x[q, n, 128k+p]; yt[q, p, k, m] = ys[q, m, 128k+p]
    xt8 = np.empty((PAIRS, 128, KT, N), f8np)
    yt8 = np.empty((PAIRS, 128, KT, N), f8np)
    for q in range(PAIRS):
        xt8[q] = (
            (x32[q].T * np.float32(-2.0)).reshape(KT, 128, N).transpose(1, 0, 2)
        ).astype(f8np)
        ys = y32[q][orders[q]]
        yt8[q] = (ys.T.reshape(KT, 128, N).transpose(1, 0, 2)).astype(f8np)

    y2grp = y2s.reshape(PAIRS, G, GS)
    cg = (y2grp.min(-1) + y2grp.max(-1)) / 2.0  # [PAIRS, G] midpoints

    # x2c[core][p, j*NT + t] = x2[q0+j, t*128 + p] - 1024  (fp32)
    x2c_pairs = (
        (x2 - 1024.0).reshape(PAIRS, NT, 128).transpose(0, 2, 1).astype(np.float32)
    )
    x2c = np.ascontiguousarray(
        x2c_pairs.reshape(N_CORES, PP, 128, NT).transpose(0, 2, 1, 3).reshape(
            N_CORES, 128, PP * NT
        )
    )
    return xt8, yt8, x2, y2s, cg, x2c


def _run(x, y, trace=False):
    global _NC_CACHE
    from concourse.bass_utils import run_bass_kernel_spmd

    xt8, yt8, x2, y2s, cg, x2c = _prep_inputs(x, y)

    if _NC_CACHE is None:
        _NC_CACHE = _build_nc()
    nc = _NC_CACHE

    in_maps = []
    for i in range(N_CORES):
        q0 = i * PP
        in_maps.append(
            {"xt": xt8[q0 : q0 + PP], "yt": yt8[q0 : q0 + PP], "x2c": x2c[i]}
        )

    res = run_bass_kernel_spmd(nc, in_maps, core_ids=list(range(N_CORES)), trace=trace)

    h2 = np.empty(PAIRS, np.float64)
    for i in range(N_CORES):
        r = res.results[i]
        for j in range(PP):
            q = i * PP + j
            # rowaccs[p, t*G+g] = min_{m in grp g}(x2 - 1024 - 2xy)
            rg = r["rowout"][j].astype(np.float64).reshape(128, NT, G)
            rn = (rg + cg[q][None, None, :]).min(-1)  # [128, NT] ~ min_m(d2)-1024
            fwd2 = rn.max() + 1024.0
            # bwd: m<512 from on-chip colacc, m>=512 from raw cc tiles
            colacc = r["colout"][j].astype(np.float64)  # [128, 512]
            lo = colacc.min(0)  # [512] = min_n(x2 - 1024 - 2xy)
            cc = r["ccout"][j].astype(np.float32).reshape(N, MBS)  # [n, m-512]
            hi = cc.min(0).astype(np.float64)
            colmin = np.concatenate([lo, hi])
            bwd2 = (colmin + y2s[q]).max() + 1024.0
            h2[q] = max(fwd2, bwd2, 0.0)

    ans = np.sqrt(h2).mean()
    return np.array(ans, dtype=np.float32), res


def kernel(input, target):
    out, _ = _run(np.asarray(input), np.asarray(target), trace=False)
    return out
